# revision 25
# baseline (speedup 1.0000x reference)
"""Trainium2 Bass kernel for a pre-LN transformer encoder layer.

Sharding: data-parallel over batch. B=8 batch elements -> 8 NeuronCores,
one full [L=1024, D=1024] encoder layer per core. No collectives.

Per-core dataflow (q = token index, d = feature index, k = key index):
  x [q,d] --LN1--> x1 [q,d] --PE transpose--> x1T [d,q] (bf16)
  V natural [k,d] (+ones col per head)  = matmul(lhsT=x1T tile, rhs=Wv rows)
  QT, KT [d,q]                          = matmul(lhsT=W col block, rhs=x1T)
  per head pair (chunk-major): ST [k,q] psum (row-packed across the two
            64-row head groups) -> ACT exp(s/8 + mask_bias) -> expS sbuf
            PV' accumulates [attnT | Z] over k tiles (ones-column trick)
            1/Z via custom-DVE approx reciprocal, replicated via a DRAM
            bounce -> attnT [d,q]
  attnproj [q,d] = matmul(lhsT=attnT tile, rhs=Wo rows); x2 = x + proj + bo
  LN2 -> x2n -> transpose -> x2nT [d,q]
  FFN1: hT [f,q] = matmul(lhsT=W1 col block, rhs=x2nT); ReLU+b1 fused in ACT
  FFN2: acc [q,d] += matmul(lhsT=hT tile, rhs=W2 rows) per f-group;
  + b2 once at the end.

Attention runs chunk-major (all 16 heads finish token-chunk 0 before
chunk 1) so the proj/LN2/FFN pipeline for the first half overlaps the
ACT-bound softmax of the second half. All matmul operands are bf16;
stats/softmax/residual arithmetic stays fp32.
"""

import numpy as np

import concourse.bass as bass
import concourse.tile as tile
from concourse import bacc, mybir
from concourse.bass import ds, ts
from concourse.masks import make_identity

B = 8
L = 1024
D = 1024
H = 16
DK = 64
F = 4096
EPS = 1e-6
NEG_INF = 1.0e9
P = 128
NQ = L // P            # 8 token tiles
ND = D // P            # 8 model-dim tiles
NF = F // P            # 32 ffn-dim tiles
CH = 512               # matmul moving free dim (one PSUM bank of fp32)
NCH = L // CH          # 2 chunks of tokens
QPC = CH // P          # 4 q-tiles per chunk
HPC = CH // DK         # 8 heads per 512-wide projection chunk
F_GROUP = 4            # f-tiles per FFN group
NG = NF // F_GROUP     # 8 groups

FP32 = mybir.dt.float32
MMD = mybir.dt.bfloat16   # matmul operand dtype
AF = mybir.ActivationFunctionType
OP = mybir.AluOpType


def build_nc():
    nc = bacc.Bacc("TRN2", target_bir_lowering=False, num_swdge_queues=4)

    xd = nc.dram_tensor("x", [L, D], FP32, kind="ExternalInput")
    maskd = nc.dram_tensor("e_mask", [1, L], mybir.dt.int32, kind="ExternalInput")
    ln1_g = nc.dram_tensor("ln1_g", [D], FP32, kind="ExternalInput")
    ln1_b = nc.dram_tensor("ln1_b", [D], FP32, kind="ExternalInput")
    wq = nc.dram_tensor("Wq", [D, D], FP32, kind="ExternalInput")
    bq = nc.dram_tensor("bq", [D], FP32, kind="ExternalInput")
    wk = nc.dram_tensor("Wk", [D, D], FP32, kind="ExternalInput")
    bk = nc.dram_tensor("bk", [D], FP32, kind="ExternalInput")
    wv = nc.dram_tensor("Wv", [D, D], FP32, kind="ExternalInput")
    bv = nc.dram_tensor("bv", [D], FP32, kind="ExternalInput")
    wo = nc.dram_tensor("Wo", [D, D], FP32, kind="ExternalInput")
    bo = nc.dram_tensor("bo", [D], FP32, kind="ExternalInput")
    ln2_g = nc.dram_tensor("ln2_g", [D], FP32, kind="ExternalInput")
    ln2_b = nc.dram_tensor("ln2_b", [D], FP32, kind="ExternalInput")
    w1 = nc.dram_tensor("W1", [D, F], FP32, kind="ExternalInput")
    b1 = nc.dram_tensor("b1", [F], FP32, kind="ExternalInput")
    w2 = nc.dram_tensor("W2", [F, D], FP32, kind="ExternalInput")
    b2 = nc.dram_tensor("b2", [D], FP32, kind="ExternalInput")
    outd = nc.dram_tensor("out", [L, D], FP32, kind="ExternalOutput")

    with tile.TileContext(nc) as tc:
        singles = tc.alloc_tile_pool(name="singles", bufs=1)
        big = tc.alloc_tile_pool(name="big", bufs=1)
        # single PSUM pool for the whole kernel: no pool-release barriers.
        # 4 (mm chains) + 2 (attention PV) + 2 (transposes) = 8 banks.
        psum = tc.alloc_tile_pool(name="psum", bufs=1, space="PSUM")

        def psum_mm():
            return psum.tile([P, CH], FP32, tag="mm", name="ps_mm", bufs=4)

        def big_tiles(shape, tagp, namep, dt=FP32):
            return [
                big.tile(shape, dt, tag=f"{tagp}{i}", name=f"{namep}{i}", bufs=1)
                for i in range(NQ)
            ]

        ident = singles.tile([P, P], MMD, name="ident")
        make_identity(nc, ident)
        eps_t = singles.tile([P, 1], FP32, name="eps_t")
        nc.vector.memset(eps_t, EPS)
        ones_h = singles.tile([P, H, 1], FP32, name="ones_h")
        nc.vector.memset(ones_h, 1.0)

        def bcast_load(pool, dram_vec, n, tag):
            """replicate a [n] DRAM vector across all 128 partitions."""
            t = pool.tile([P, n], FP32, tag=tag, name=tag, bufs=1)
            src = bass.AP(
                tensor=dram_vec.tensor,
                offset=dram_vec.offset,
                ap=[[0, P], [1, n]],
            )
            nc.sync.dma_start(out=t, in_=src)
            return t

        def col_load(dram_vec, ntiles, name):
            """[ntiles*128] DRAM vector -> [128, ntiles], col t = v[t*128:+128]."""
            t = singles.tile([P, ntiles], FP32, name=name)
            nc.sync.dma_start(out=t, in_=dram_vec.rearrange("(t p) -> p t", p=P))
            return t

        bq_c = col_load(bq.ap(), ND, "bq_c")
        bk_c = col_load(bk.ap(), ND, "bk_c")
        b1_c = col_load(b1.ap(), NF, "b1_c")

        # additive attention-mask bias per key position: (mask-1)*NEG_INF
        mask_i = singles.tile([P, NQ], mybir.dt.int32, name="mask_i")
        nc.sync.dma_start(out=mask_i, in_=maskd.ap()[0].rearrange("(t p) -> p t", p=P))
        mask_f = singles.tile([P, NQ], FP32, name="mask_f")
        nc.vector.tensor_copy(out=mask_f, in_=mask_i)
        ebias = singles.tile([P, NQ], FP32, name="ebias")
        nc.vector.tensor_scalar(
            out=ebias, in0=mask_f, scalar1=1.0, scalar2=NEG_INF,
            op0=OP.subtract, op1=OP.mult,
        )

        def layer_norm_tile(pool, x_t, g_bc, b_bc):
            stats = pool.tile([P, 2, 6], FP32, tag="ln_stats", name="ln_stats")
            xr = x_t.rearrange("p (s c) -> p s c", s=2)
            for s in range(2):
                nc.vector.bn_stats(out=stats[:, s, :], in_=xr[:, s, :])
            mv = pool.tile([P, 2], FP32, tag="ln_mv", name="ln_mv")
            nc.vector.bn_aggr(out=mv, in_=stats)
            rstd = pool.tile([P, 1], FP32, tag="ln_rstd", name="ln_rstd")
            nc.scalar.activation(out=rstd, in_=mv[:, 1:2], func=AF.Sqrt,
                                 bias=eps_t, scale=1.0)
            nc.vector.reciprocal(out=rstd, in_=rstd)
            xn = pool.tile([P, D], MMD, tag="ln_out", name="ln_out")
            xf = pool.tile([P, D], FP32, tag="ln_f32", name="ln_f32", bufs=1)
            nc.vector.tensor_scalar(
                out=xf, in0=x_t, scalar1=mv[:, 0:1], scalar2=rstd,
                op0=OP.subtract, op1=OP.mult,
            )
            nc.vector.tensor_mul(out=xf, in0=xf, in1=g_bc)
            nc.vector.tensor_add(out=xn, in0=xf, in1=b_bc)
            return xn

        def transpose_into(src_tile, qt, dst_tiles):
            """src natural [P, D] bf16 tile (token tile qt) -> dst [d,q] cols."""
            for dt in range(ND):
                pt = psum.tile([P, P], MMD, tag="tp", name="tp", bufs=2)
                nc.tensor.transpose(pt, src_tile[:, ts(dt, P)], ident)
                nc.vector.tensor_copy(out=dst_tiles[dt][:, ts(qt, P)], in_=pt)

        # persistent activations (tag groups; A is reused by x2nT later)
        x1T = big_tiles([P, L], "A", "x1T", MMD)
        qT = big_tiles([P, L], "B", "qT", MMD)
        kT = big_tiles([P, L], "C", "kT", MMD)
        attnT = big_tiles([P, L], "AT", "attnT", MMD)
        vn = [
            big.tile([P, H, DK + 1], MMD, tag=f"V{i}", name=f"vn{i}", bufs=1)
            for i in range(NQ)
        ]
        x2 = big_tiles([P, D], "X2", "x2", FP32)

        # ---------- phase 1: LN1 + transpose ----------
        with tc.tile_pool(name="ph1", bufs=3) as ph1:
            g1_bc = bcast_load(ph1, ln1_g.ap(), D, "g1")
            b1ln_bc = bcast_load(ph1, ln1_b.ap(), D, "b1ln")
            for qt in range(NQ):
                x_t = ph1.tile([P, D], FP32, tag="x_in", name="x_in")
                nc.sync.dma_start(out=x_t, in_=xd.ap()[ts(qt, P), :])
                x1 = layer_norm_tile(ph1, x_t, g1_bc, b1ln_bc)
                transpose_into(x1, qt, x1T)

        # Wo rows + bo prefetched now; consumed in phase 4
        ph4w = tc.alloc_tile_pool(name="ph4w", bufs=1)
        bo_bc = bcast_load(ph4w, bo.ap(), D, "bo_bc")
        wo_rows = []
        for dt in range(ND):
            wt = ph4w.tile([P, D], MMD, tag=f"wo_row{dt}",
                           name=f"wo_row{dt}", bufs=1)
            nc.gpsimd.dma_start(out=wt, in_=wo.ap()[ts(dt, P), :])
            wo_rows.append(wt)

        # ---------- phase 2: V natural (+ones col) ----------
        with tc.tile_pool(name="ph2v", bufs=1) as ph2v:
            bv_bc = bcast_load(ph2v, bv.ap(), D, "bv_bc")
            for qt in range(NQ):
                nc.vector.tensor_copy(out=vn[qt][:, :, DK:DK + 1], in_=ones_h)
            wv_rows = []
            for dt in range(ND):
                wt = ph2v.tile([P, D], MMD, tag=f"wv_row{dt}",
                               name=f"wv_row{dt}", bufs=1)
                nc.gpsimd.dma_start(out=wt, in_=wv.ap()[ts(dt, P), :])
                wv_rows.append(wt)
            for qt in range(NQ):
                for ch in range(NCH):
                    ps = psum_mm()
                    for dt in range(ND):
                        nc.tensor.matmul(
                            ps, x1T[dt][:, ts(qt, P)],
                            wv_rows[dt][:, ts(ch, CH)],
                            start=(dt == 0), stop=(dt == ND - 1),
                        )
                    nc.vector.scalar_tensor_tensor(
                        out=vn[qt][:, ds(ch * HPC, HPC), 0:DK],
                        in0=ps.rearrange("p (h d) -> p h d", d=DK),
                        scalar=0.0,
                        in1=bv_bc[:, ts(ch, CH)].rearrange("p (h d) -> p h d", d=DK),
                        op0=OP.add, op1=OP.add,
                    )

        # ---------- phases 3: QK + attention (chunk-major) ----------
        with tc.tile_pool(name="ph3", bufs=3) as ph3, \
             tc.tile_pool(name="ph3w", bufs=2) as ph3w, \
             tc.tile_pool(name="ph3d", bufs=3, space="DRAM") as ph3d:

            def emit_attention_pair_chunk(dt, ch):
                """S (row-packed across both heads of d-tile dt), exp, and
                the PV' accumulation step per k-tile, for token chunk ch.
                Streaming expS per k-tile keeps the S->exp->PV chain deep in
                flight with only [P, CH]-sized softmax buffers."""
                heads = (2 * dt, 2 * dt + 1)
                pa = {
                    h: psum.tile([P, CH], FP32, tag=f"pv{h % 2}",
                                 name="ps_a", bufs=1)
                    for h in heads
                }
                for kt in range(NQ):
                    es = {}
                    for h in heads:
                        rbase = (h % 2) * DK
                        ps = psum_mm()
                        nc.tensor.matmul(
                            ps,
                            kT[dt][rbase:rbase + DK, ts(kt, P)],
                            qT[dt][rbase:rbase + DK, ts(ch, CH)],
                            start=True, stop=True,
                        )
                        e = ph3.tile([P, CH], MMD, tag=f"expS{h % 2}",
                                     name="expS", bufs=3)
                        nc.scalar.activation(
                            out=e, in_=ps, func=AF.Exp,
                            bias=ebias[:, kt:kt + 1], scale=0.125,
                        )
                        es[h] = e
                    for h in heads:
                        nc.tensor.matmul(
                            pa[h][0:DK + 1, :],
                            vn[kt][:, h, :],
                            es[h],
                            start=(kt == 0), stop=(kt == NQ - 1),
                        )
                for h in heads:
                    rbase = (h % 2) * DK
                    # decouple the tail so the PV psum recycles after one copy
                    pv_sb = ph3.tile([P, CH], FP32, tag="pv_sb", name="pv_sb",
                                     bufs=2)
                    nc.vector.tensor_copy(out=pv_sb[0:DK + 1, :],
                                          in_=pa[h][0:DK + 1, :])
                    # ~51-ULP reciprocal of the Z row (full-tile custom-DVE
                    # op; sliced APs mislower). Replicate Z across partitions
                    # via a DRAM bounce (SBUF DMA sources need nonzero
                    # partition step, DRAM sources don't).
                    rzrow = ph3.tile([P, CH], FP32, tag="rzrow", name="rzrow", bufs=1)
                    nc.vector.reciprocal_approx_fast(out=rzrow, in_=pv_sb)
                    zscr = ph3d.tile([1, CH], FP32, tag="zscr", name="zscr")
                    nc.sync.dma_start(out=zscr, in_=rzrow[DK:DK + 1, :])
                    rzb = ph3.tile([DK, CH], FP32, tag="rzb", name="rzb", bufs=2)
                    nc.sync.dma_start(
                        out=rzb,
                        in_=bass.AP(
                            tensor=zscr.tensor, offset=zscr.offset,
                            ap=[[0, DK], [1, CH]],
                        ),
                    )
                    attn_h = ph3.tile([DK, CH], MMD, tag="attn_h", name="attn_h", bufs=2)
                    nc.vector.tensor_mul(out=attn_h, in0=pv_sb[0:DK, :], in1=rzb)
                    nc.sync.dma_start(
                        out=attnT[dt][rbase:rbase + DK, ts(ch, CH)], in_=attn_h
                    )

            for dt_out in range(ND):
                for (wmat, bias_c, dstT) in ((wq, bq_c, qT), (wk, bk_c, kT)):
                    wt = ph3w.tile([P, ND, P], MMD, tag="w_col", name="w_col")
                    nc.gpsimd.dma_start(
                        out=wt,
                        in_=wmat.ap().rearrange("(a p) b -> p a b", p=P)[
                            :, :, ts(dt_out, P)],
                    )
                    for ch in range(NCH):
                        ps = psum_mm()
                        for dt_in in range(ND):
                            nc.tensor.matmul(
                                ps, wt[:, dt_in, :],
                                x1T[dt_in][:, ts(ch, CH)],
                                start=(dt_in == 0), stop=(dt_in == ND - 1),
                            )
                        nc.scalar.activation(
                            out=dstT[dt_out][:, ts(ch, CH)], in_=ps,
                            func=AF.Identity, bias=bias_c[:, dt_out:dt_out + 1],
                            scale=1.0,
                        )
                emit_attention_pair_chunk(dt_out, 0)
            for dt_out in range(ND):
                emit_attention_pair_chunk(dt_out, 1)

            # ---------- phase 4+5: out-proj + residual + LN2 + transpose ----
            # Emitted inside the ph3 scope, chunk-major, so chunk-0 proj/LN2
            # overlaps the chunk-1 attention still in flight.
            x2nT = big_tiles([P, L], "A", "x2nT", MMD)  # reuses x1T slots
            with tc.tile_pool(name="ph4", bufs=2) as ph4:
                g2_bc = bcast_load(ph4, ln2_g.ap(), D, "g2")
                b2ln_bc = bcast_load(ph4, ln2_b.ap(), D, "b2ln")
                for ch in range(NCH):
                    for qi in range(QPC):
                        qt = ch * QPC + qi
                        x_t = ph4.tile([P, D], FP32, tag="x_again", name="x_again")
                        nc.sync.dma_start(out=x_t, in_=xd.ap()[ts(qt, P), :])
                        for oc in range(NCH):
                            ps = psum_mm()
                            for dt in range(ND):
                                nc.tensor.matmul(
                                    ps, attnT[dt][:, ts(qt, P)],
                                    wo_rows[dt][:, ts(oc, CH)],
                                    start=(dt == 0), stop=(dt == ND - 1),
                                )
                            nc.vector.tensor_add(
                                out=x2[qt][:, ts(oc, CH)], in0=ps,
                                in1=x_t[:, ts(oc, CH)],
                            )
                        nc.vector.tensor_add(out=x2[qt], in0=x2[qt], in1=bo_bc)
                        x2n = layer_norm_tile(ph4, x2[qt], g2_bc, b2ln_bc)
                        transpose_into(x2n, qt, x2nT)

        ph4w.release()

        # ---------- phase 6: FFN ----------
        acc = [
            big.tile([P, D], FP32, tag=f"V{i}", name=f"acc{i}", bufs=1)
            for i in range(NQ)
        ]

        with tc.tile_pool(name="ph6", bufs=1) as ph6, \
             tc.tile_pool(name="ph6w", bufs=2) as ph6w, \
             tc.tile_pool(name="ph6h", bufs=1) as ph6h:
            ones_row = ph6.tile([1, P], MMD, tag="ones_row", name="ones_row",
                                bufs=1)
            nc.vector.memset(ones_row, 1.0)
            b2_row = ph6.tile([1, D], MMD, tag="b2_row", name="b2_row", bufs=1)
            nc.gpsimd.dma_start(out=b2_row, in_=b2.ap().unsqueeze(0))
            w1r = w1.ap().rearrange("(a p) b -> p a b", p=P)
            for g in range(NG):
                hts = []
                w2_rows = []
                for fi in range(F_GROUP):
                    ft = g * F_GROUP + fi
                    w1t = ph6w.tile([P, ND, P], MMD, tag="w1_col", name="w1_col", bufs=4)
                    nc.gpsimd.dma_start(out=w1t, in_=w1r[:, :, ts(ft, P)])
                    w2t = ph6w.tile([P, D], MMD, tag=f"w2_row{fi}",
                                    name=f"w2_row{fi}", bufs=2)
                    nc.gpsimd.dma_start(out=w2t, in_=w2.ap()[ts(ft, P), :])
                    w2_rows.append(w2t)
                    ht = ph6h.tile([P, L], MMD, tag=f"ht{fi}",
                                   name=f"ht{fi}", bufs=2)
                    for ch in range(NCH):
                        ps = psum_mm()
                        for dt in range(ND):
                            nc.tensor.matmul(
                                ps, w1t[:, dt, :],
                                x2nT[dt][:, ts(ch, CH)],
                                start=(dt == 0), stop=(dt == ND - 1),
                            )
                        nc.scalar.activation(
                            out=ht[:, ts(ch, CH)], in_=ps, func=AF.Relu,
                            bias=b1_c[:, ft:ft + 1], scale=1.0,
                        )
                    hts.append(ht)
                for qt in range(NQ):
                    for ch in range(NCH):
                        ps = psum_mm()
                        for fi in range(F_GROUP):
                            nc.tensor.matmul(
                                ps, hts[fi][:, ts(qt, P)],
                                w2_rows[fi][:, ts(ch, CH)],
                                start=(fi == 0),
                                stop=(fi == F_GROUP - 1 and g != 0),
                            )
                        if g == 0:
                            # fold the fc2 bias in as a K=1 broadcast matmul
                            nc.tensor.matmul(
                                ps, ones_row, b2_row[:, ts(ch, CH)],
                                start=False, stop=True,
                            )
                            # and the residual stream via the copy-out add
                            nc.vector.tensor_add(
                                out=acc[qt][:, ts(ch, CH)],
                                in0=ps, in1=x2[qt][:, ts(ch, CH)],
                            )
                        else:
                            nc.vector.tensor_add(
                                out=acc[qt][:, ts(ch, CH)],
                                in0=acc[qt][:, ts(ch, CH)], in1=ps,
                            )
                        if g == NG - 1:
                            # acc[qt] chunk finalized: store immediately
                            nc.sync.dma_start(
                                out=outd.ap()[ts(qt, P), ts(ch, CH)],
                                in_=acc[qt][:, ts(ch, CH)],
                            )

        psum.release()
        big.release()
        singles.release()

    nc.finalize()
    return nc


_NC_CACHE = None


def _get_nc():
    global _NC_CACHE
    if _NC_CACHE is None:
        _NC_CACHE = build_nc()
    return _NC_CACHE


def run(inputs, trace=False):
    """Run on 8 cores; returns (out [8,L,D], BassKernelResults)."""
    from concourse.bass_utils import run_bass_kernel_spmd

    nc = _get_nc()
    weights = {
        k: np.ascontiguousarray(np.asarray(inputs[k], dtype=np.float32))
        for k in ("ln1_g", "ln1_b", "Wq", "bq", "Wk", "bk", "Wv", "bv",
                  "Wo", "bo", "ln2_g", "ln2_b", "W1", "b1", "W2", "b2")
    }
    x = np.asarray(inputs["x"], dtype=np.float32)
    e_mask = np.asarray(inputs["e_mask"], dtype=np.int32)
    in_maps = []
    for b in range(B):
        m = dict(weights)
        m["x"] = np.ascontiguousarray(x[b])
        m["e_mask"] = np.ascontiguousarray(e_mask[b])
        in_maps.append(m)
    res = run_bass_kernel_spmd(nc, in_maps, core_ids=list(range(B)), trace=trace)
    out = np.stack([res.results[b]["out"] for b in range(B)], axis=0)
    return out, res


def kernel(**inputs):
    out, _ = run(inputs, trace=False)
    return out


# revision 26
# speedup vs baseline: 1.0236x; 1.0236x over previous
"""Trainium2 Bass kernel for a pre-LN transformer encoder layer.

Sharding: data-parallel over batch. B=8 batch elements -> 8 NeuronCores,
one full [L=1024, D=1024] encoder layer per core. No collectives.

Per-core dataflow (q = token index, d = feature index, k = key index):
  x [q,d] --LN1--> x1 [q,d] --PE transpose--> x1T [d,q] (bf16)
  V natural [k,d] (+ones col per head)  = matmul(lhsT=x1T tile, rhs=Wv rows)
  QT, KT [d,q]                          = matmul(lhsT=W col block, rhs=x1T)
  per head pair (chunk-major): ST [k,q] psum (row-packed across the two
            64-row head groups) -> ACT exp(s/8 + mask_bias) -> expS sbuf
            PV' accumulates [attnT | Z] over k tiles (ones-column trick)
            1/Z via custom-DVE approx reciprocal, replicated via a DRAM
            bounce -> attnT [d,q]
  attnproj [q,d] = matmul(lhsT=attnT tile, rhs=Wo rows); x2 = x + proj + bo
  LN2 -> x2n -> transpose -> x2nT [d,q]
  FFN1: hT [f,q] = matmul(lhsT=W1 col block, rhs=x2nT); ReLU+b1 fused in ACT
  FFN2: acc [q,d] += matmul(lhsT=hT tile, rhs=W2 rows) per f-group;
  + b2 once at the end.

Attention runs chunk-major (all 16 heads finish token-chunk 0 before
chunk 1) so the proj/LN2/FFN pipeline for the first half overlaps the
ACT-bound softmax of the second half. All matmul operands are bf16;
stats/softmax/residual arithmetic stays fp32.
"""

import numpy as np

import concourse.bass as bass
import concourse.tile as tile
from concourse import bacc, mybir
from concourse.bass import ds, ts
from concourse.masks import make_identity

B = 8
L = 1024
D = 1024
H = 16
DK = 64
F = 4096
EPS = 1e-6
NEG_INF = 1.0e9
P = 128
NQ = L // P            # 8 token tiles
ND = D // P            # 8 model-dim tiles
NF = F // P            # 32 ffn-dim tiles
CH = 512               # matmul moving free dim (one PSUM bank of fp32)
NCH = L // CH          # 2 chunks of tokens
QPC = CH // P          # 4 q-tiles per chunk
HPC = CH // DK         # 8 heads per 512-wide projection chunk
F_GROUP = 4            # f-tiles per FFN group
NG = NF // F_GROUP     # 8 groups

FP32 = mybir.dt.float32
MMD = mybir.dt.bfloat16   # matmul operand dtype
AF = mybir.ActivationFunctionType
OP = mybir.AluOpType


def build_nc():
    nc = bacc.Bacc("TRN2", target_bir_lowering=False, num_swdge_queues=4)

    xd = nc.dram_tensor("x", [L, D], FP32, kind="ExternalInput")
    maskd = nc.dram_tensor("e_mask", [1, L], mybir.dt.int32, kind="ExternalInput")
    ln1_g = nc.dram_tensor("ln1_g", [D], FP32, kind="ExternalInput")
    ln1_b = nc.dram_tensor("ln1_b", [D], FP32, kind="ExternalInput")
    wq = nc.dram_tensor("Wq", [D, D], FP32, kind="ExternalInput")
    bq = nc.dram_tensor("bq", [D], FP32, kind="ExternalInput")
    wk = nc.dram_tensor("Wk", [D, D], FP32, kind="ExternalInput")
    bk = nc.dram_tensor("bk", [D], FP32, kind="ExternalInput")
    wv = nc.dram_tensor("Wv", [D, D], FP32, kind="ExternalInput")
    bv = nc.dram_tensor("bv", [D], FP32, kind="ExternalInput")
    wo = nc.dram_tensor("Wo", [D, D], FP32, kind="ExternalInput")
    bo = nc.dram_tensor("bo", [D], FP32, kind="ExternalInput")
    ln2_g = nc.dram_tensor("ln2_g", [D], FP32, kind="ExternalInput")
    ln2_b = nc.dram_tensor("ln2_b", [D], FP32, kind="ExternalInput")
    w1 = nc.dram_tensor("W1", [D, F], FP32, kind="ExternalInput")
    b1 = nc.dram_tensor("b1", [F], FP32, kind="ExternalInput")
    w2 = nc.dram_tensor("W2", [F, D], FP32, kind="ExternalInput")
    b2 = nc.dram_tensor("b2", [D], FP32, kind="ExternalInput")
    outd = nc.dram_tensor("out", [L, D], FP32, kind="ExternalOutput")

    with tile.TileContext(nc) as tc:
        singles = tc.alloc_tile_pool(name="singles", bufs=1)
        big = tc.alloc_tile_pool(name="big", bufs=1)
        # single PSUM pool for the whole kernel: no pool-release barriers.
        # 4 (mm chains) + 2 (attention PV) + 2 (transposes) = 8 banks.
        psum = tc.alloc_tile_pool(name="psum", bufs=1, space="PSUM")

        def psum_mm():
            return psum.tile([P, CH], FP32, tag="mm", name="ps_mm", bufs=4)

        def big_tiles(shape, tagp, namep, dt=FP32):
            return [
                big.tile(shape, dt, tag=f"{tagp}{i}", name=f"{namep}{i}", bufs=1)
                for i in range(NQ)
            ]

        ident = singles.tile([P, P], MMD, name="ident")
        make_identity(nc, ident)
        eps_t = singles.tile([P, 1], FP32, name="eps_t")
        nc.vector.memset(eps_t, EPS)
        ones_h = singles.tile([P, H, 1], FP32, name="ones_h")
        nc.vector.memset(ones_h, 1.0)

        def bcast_load(pool, dram_vec, n, tag):
            """replicate a [n] DRAM vector across all 128 partitions."""
            t = pool.tile([P, n], FP32, tag=tag, name=tag, bufs=1)
            src = bass.AP(
                tensor=dram_vec.tensor,
                offset=dram_vec.offset,
                ap=[[0, P], [1, n]],
            )
            nc.sync.dma_start(out=t, in_=src)
            return t

        def col_load(dram_vec, ntiles, name):
            """[ntiles*128] DRAM vector -> [128, ntiles], col t = v[t*128:+128]."""
            t = singles.tile([P, ntiles], FP32, name=name)
            nc.sync.dma_start(out=t, in_=dram_vec.rearrange("(t p) -> p t", p=P))
            return t

        bq_c = col_load(bq.ap(), ND, "bq_c")
        bk_c = col_load(bk.ap(), ND, "bk_c")
        b1_c = col_load(b1.ap(), NF, "b1_c")

        # additive attention-mask bias per key position: (mask-1)*NEG_INF
        mask_i = singles.tile([P, NQ], mybir.dt.int32, name="mask_i")
        nc.sync.dma_start(out=mask_i, in_=maskd.ap()[0].rearrange("(t p) -> p t", p=P))
        mask_f = singles.tile([P, NQ], FP32, name="mask_f")
        nc.vector.tensor_copy(out=mask_f, in_=mask_i)
        ebias = singles.tile([P, NQ], FP32, name="ebias")
        nc.vector.tensor_scalar(
            out=ebias, in0=mask_f, scalar1=1.0, scalar2=NEG_INF,
            op0=OP.subtract, op1=OP.mult,
        )

        def layer_norm_tile(pool, x_t, g_bc, b_bc):
            stats = pool.tile([P, 2, 6], FP32, tag="ln_stats", name="ln_stats")
            xr = x_t.rearrange("p (s c) -> p s c", s=2)
            for s in range(2):
                nc.vector.bn_stats(out=stats[:, s, :], in_=xr[:, s, :])
            mv = pool.tile([P, 2], FP32, tag="ln_mv", name="ln_mv")
            nc.vector.bn_aggr(out=mv, in_=stats)
            rstd = pool.tile([P, 1], FP32, tag="ln_rstd", name="ln_rstd")
            nc.scalar.activation(out=rstd, in_=mv[:, 1:2], func=AF.Sqrt,
                                 bias=eps_t, scale=1.0)
            nc.vector.reciprocal(out=rstd, in_=rstd)
            xn = pool.tile([P, D], MMD, tag="ln_out", name="ln_out")
            xf = pool.tile([P, D], FP32, tag="ln_f32", name="ln_f32", bufs=1)
            nc.vector.tensor_scalar(
                out=xf, in0=x_t, scalar1=mv[:, 0:1], scalar2=rstd,
                op0=OP.subtract, op1=OP.mult,
            )
            nc.vector.tensor_mul(out=xf, in0=xf, in1=g_bc)
            nc.vector.tensor_add(out=xn, in0=xf, in1=b_bc)
            return xn

        def transpose_into(src_tile, qt, dst_tiles):
            """src natural [P, D] bf16 tile (token tile qt) -> dst [d,q] cols."""
            for dt in range(ND):
                pt = psum.tile([P, P], MMD, tag="tp", name="tp", bufs=2)
                nc.tensor.transpose(pt, src_tile[:, ts(dt, P)], ident)
                nc.vector.tensor_copy(out=dst_tiles[dt][:, ts(qt, P)], in_=pt)

        # persistent activations (tag groups; A is reused by x2nT later)
        x1T = big_tiles([P, L], "A", "x1T", MMD)
        qT = big_tiles([P, L], "B", "qT", MMD)
        kT = big_tiles([P, L], "C", "kT", MMD)
        attnT = big_tiles([P, L], "AT", "attnT", MMD)
        vn = [
            big.tile([P, H, DK + 1], MMD, tag=f"V{i}", name=f"vn{i}", bufs=1)
            for i in range(NQ)
        ]
        x2 = big_tiles([P, D], "X2", "x2", FP32)

        # weight prefetch pools allocated BEFORE phase 1 so the casting
        # DMAs start immediately (allocating them later would reuse ph1's
        # addresses and false-depend on LN1 finishing)
        ph4w = tc.alloc_tile_pool(name="ph4w", bufs=1)
        bo_bc = bcast_load(ph4w, bo.ap(), D, "bo_bc")
        wo_rows = []
        for dt in range(ND):
            wt = ph4w.tile([P, D], MMD, tag=f"wo_row{dt}",
                           name=f"wo_row{dt}", bufs=1)
            nc.gpsimd.dma_start(out=wt, in_=wo.ap()[ts(dt, P), :])
            wo_rows.append(wt)
        ph2v = tc.alloc_tile_pool(name="ph2v", bufs=1)
        bv_bc = bcast_load(ph2v, bv.ap(), D, "bv_bc")
        wv_rows = []
        for dt in range(ND):
            wt = ph2v.tile([P, D], MMD, tag=f"wv_row{dt}",
                           name=f"wv_row{dt}", bufs=1)
            nc.gpsimd.dma_start(out=wt, in_=wv.ap()[ts(dt, P), :])
            wv_rows.append(wt)
        for qt in range(NQ):
            nc.vector.tensor_copy(out=vn[qt][:, :, DK:DK + 1], in_=ones_h)

        # ---------- phase 1: LN1 + transpose ----------
        with tc.tile_pool(name="ph1", bufs=3) as ph1:
            g1_bc = bcast_load(ph1, ln1_g.ap(), D, "g1")
            b1ln_bc = bcast_load(ph1, ln1_b.ap(), D, "b1ln")
            for qt in range(NQ):
                x_t = ph1.tile([P, D], FP32, tag="x_in", name="x_in")
                nc.sync.dma_start(out=x_t, in_=xd.ap()[ts(qt, P), :])
                x1 = layer_norm_tile(ph1, x_t, g1_bc, b1ln_bc)
                transpose_into(x1, qt, x1T)

        # ---------- phase 2: V natural (+ones col) ----------
        if True:
            for qt in range(NQ):
                for ch in range(NCH):
                    ps = psum_mm()
                    for dt in range(ND):
                        nc.tensor.matmul(
                            ps, x1T[dt][:, ts(qt, P)],
                            wv_rows[dt][:, ts(ch, CH)],
                            start=(dt == 0), stop=(dt == ND - 1),
                        )
                    nc.vector.scalar_tensor_tensor(
                        out=vn[qt][:, ds(ch * HPC, HPC), 0:DK],
                        in0=ps.rearrange("p (h d) -> p h d", d=DK),
                        scalar=0.0,
                        in1=bv_bc[:, ts(ch, CH)].rearrange("p (h d) -> p h d", d=DK),
                        op0=OP.add, op1=OP.add,
                    )

        ph2v.release()

        # ---------- phases 3: QK + attention (chunk-major) ----------
        with tc.tile_pool(name="ph3", bufs=3) as ph3, \
             tc.tile_pool(name="ph3w", bufs=2) as ph3w, \
             tc.tile_pool(name="ph3d", bufs=3, space="DRAM") as ph3d:

            def emit_attention_pair_chunk(dt, ch):
                """S (row-packed across both heads of d-tile dt), exp, and
                the PV' accumulation step per k-tile, for token chunk ch.
                Streaming expS per k-tile keeps the S->exp->PV chain deep in
                flight with only [P, CH]-sized softmax buffers."""
                heads = (2 * dt, 2 * dt + 1)
                pa = {
                    h: psum.tile([P, CH], FP32, tag=f"pv{h % 2}",
                                 name="ps_a", bufs=1)
                    for h in heads
                }
                for kt in range(NQ):
                    es = {}
                    for h in heads:
                        rbase = (h % 2) * DK
                        ps = psum_mm()
                        nc.tensor.matmul(
                            ps,
                            kT[dt][rbase:rbase + DK, ts(kt, P)],
                            qT[dt][rbase:rbase + DK, ts(ch, CH)],
                            start=True, stop=True,
                        )
                        e = ph3.tile([P, CH], MMD, tag=f"expS{h % 2}",
                                     name="expS", bufs=3)
                        nc.scalar.activation(
                            out=e, in_=ps, func=AF.Exp,
                            bias=ebias[:, kt:kt + 1], scale=0.125,
                        )
                        es[h] = e
                    for h in heads:
                        nc.tensor.matmul(
                            pa[h][0:DK + 1, :],
                            vn[kt][:, h, :],
                            es[h],
                            start=(kt == 0), stop=(kt == NQ - 1),
                        )
                for h in heads:
                    rbase = (h % 2) * DK
                    # decouple the tail so the PV psum recycles after one copy
                    pv_sb = ph3.tile([P, CH], FP32, tag="pv_sb", name="pv_sb",
                                     bufs=2)
                    nc.vector.tensor_copy(out=pv_sb[0:DK + 1, :],
                                          in_=pa[h][0:DK + 1, :])
                    # ~51-ULP reciprocal of the Z row (full-tile custom-DVE
                    # op; sliced APs mislower). Replicate Z across partitions
                    # via a DRAM bounce (SBUF DMA sources need nonzero
                    # partition step, DRAM sources don't).
                    rzrow = ph3.tile([P, CH], FP32, tag="rzrow", name="rzrow", bufs=1)
                    nc.vector.reciprocal_approx_fast(out=rzrow, in_=pv_sb)
                    zscr = ph3d.tile([1, CH], FP32, tag="zscr", name="zscr")
                    nc.sync.dma_start(out=zscr, in_=rzrow[DK:DK + 1, :])
                    rzb = ph3.tile([DK, CH], FP32, tag="rzb", name="rzb", bufs=2)
                    nc.sync.dma_start(
                        out=rzb,
                        in_=bass.AP(
                            tensor=zscr.tensor, offset=zscr.offset,
                            ap=[[0, DK], [1, CH]],
                        ),
                    )
                    attn_h = ph3.tile([DK, CH], MMD, tag="attn_h", name="attn_h", bufs=2)
                    nc.vector.tensor_mul(out=attn_h, in0=pv_sb[0:DK, :], in1=rzb)
                    nc.sync.dma_start(
                        out=attnT[dt][rbase:rbase + DK, ts(ch, CH)], in_=attn_h
                    )

            for dt_out in range(ND):
                for (wmat, bias_c, dstT) in ((wq, bq_c, qT), (wk, bk_c, kT)):
                    wt = ph3w.tile([P, ND, P], MMD, tag="w_col", name="w_col")
                    nc.gpsimd.dma_start(
                        out=wt,
                        in_=wmat.ap().rearrange("(a p) b -> p a b", p=P)[
                            :, :, ts(dt_out, P)],
                    )
                    for ch in range(NCH):
                        ps = psum_mm()
                        for dt_in in range(ND):
                            nc.tensor.matmul(
                                ps, wt[:, dt_in, :],
                                x1T[dt_in][:, ts(ch, CH)],
                                start=(dt_in == 0), stop=(dt_in == ND - 1),
                            )
                        nc.scalar.activation(
                            out=dstT[dt_out][:, ts(ch, CH)], in_=ps,
                            func=AF.Identity, bias=bias_c[:, dt_out:dt_out + 1],
                            scale=1.0,
                        )
                emit_attention_pair_chunk(dt_out, 0)
            for dt_out in range(ND):
                emit_attention_pair_chunk(dt_out, 1)

            # ---------- phase 4+5: out-proj + residual + LN2 + transpose ----
            # Emitted inside the ph3 scope, chunk-major, so chunk-0 proj/LN2
            # overlaps the chunk-1 attention still in flight.
            x2nT = big_tiles([P, L], "A", "x2nT", MMD)  # reuses x1T slots
            with tc.tile_pool(name="ph4", bufs=2) as ph4:
                g2_bc = bcast_load(ph4, ln2_g.ap(), D, "g2")
                b2ln_bc = bcast_load(ph4, ln2_b.ap(), D, "b2ln")
                for ch in range(NCH):
                    for qi in range(QPC):
                        qt = ch * QPC + qi
                        x_t = ph4.tile([P, D], FP32, tag="x_again", name="x_again")
                        nc.sync.dma_start(out=x_t, in_=xd.ap()[ts(qt, P), :])
                        for oc in range(NCH):
                            ps = psum_mm()
                            for dt in range(ND):
                                nc.tensor.matmul(
                                    ps, attnT[dt][:, ts(qt, P)],
                                    wo_rows[dt][:, ts(oc, CH)],
                                    start=(dt == 0), stop=(dt == ND - 1),
                                )
                            nc.vector.tensor_add(
                                out=x2[qt][:, ts(oc, CH)], in0=ps,
                                in1=x_t[:, ts(oc, CH)],
                            )
                        nc.vector.tensor_add(out=x2[qt], in0=x2[qt], in1=bo_bc)
                        x2n = layer_norm_tile(ph4, x2[qt], g2_bc, b2ln_bc)
                        transpose_into(x2n, qt, x2nT)

        ph4w.release()

        # ---------- phase 6: FFN ----------
        acc = [
            big.tile([P, D], FP32, tag=f"V{i}", name=f"acc{i}", bufs=1)
            for i in range(NQ)
        ]

        with tc.tile_pool(name="ph6", bufs=1) as ph6, \
             tc.tile_pool(name="ph6w", bufs=2) as ph6w, \
             tc.tile_pool(name="ph6h", bufs=1) as ph6h:
            ones_row = ph6.tile([1, P], MMD, tag="ones_row", name="ones_row",
                                bufs=1)
            nc.vector.memset(ones_row, 1.0)
            b2_row = ph6.tile([1, D], MMD, tag="b2_row", name="b2_row", bufs=1)
            nc.gpsimd.dma_start(out=b2_row, in_=b2.ap().unsqueeze(0))
            w1r = w1.ap().rearrange("(a p) b -> p a b", p=P)
            for g in range(NG):
                hts = []
                w2_rows = []
                for fi in range(F_GROUP):
                    ft = g * F_GROUP + fi
                    w1t = ph6w.tile([P, ND, P], MMD, tag="w1_col", name="w1_col", bufs=4)
                    nc.gpsimd.dma_start(out=w1t, in_=w1r[:, :, ts(ft, P)])
                    w2t = ph6w.tile([P, D], MMD, tag=f"w2_row{fi}",
                                    name=f"w2_row{fi}", bufs=2)
                    nc.gpsimd.dma_start(out=w2t, in_=w2.ap()[ts(ft, P), :])
                    w2_rows.append(w2t)
                    ht = ph6h.tile([P, L], MMD, tag=f"ht{fi}",
                                   name=f"ht{fi}", bufs=2)
                    for ch in range(NCH):
                        ps = psum_mm()
                        for dt in range(ND):
                            nc.tensor.matmul(
                                ps, w1t[:, dt, :],
                                x2nT[dt][:, ts(ch, CH)],
                                start=(dt == 0), stop=(dt == ND - 1),
                            )
                        nc.scalar.activation(
                            out=ht[:, ts(ch, CH)], in_=ps, func=AF.Relu,
                            bias=b1_c[:, ft:ft + 1], scale=1.0,
                        )
                    hts.append(ht)
                for qt in range(NQ):
                    for ch in range(NCH):
                        ps = psum_mm()
                        for fi in range(F_GROUP):
                            nc.tensor.matmul(
                                ps, hts[fi][:, ts(qt, P)],
                                w2_rows[fi][:, ts(ch, CH)],
                                start=(fi == 0),
                                stop=(fi == F_GROUP - 1 and g != 0),
                            )
                        if g == 0:
                            # fold the fc2 bias in as a K=1 broadcast matmul
                            nc.tensor.matmul(
                                ps, ones_row, b2_row[:, ts(ch, CH)],
                                start=False, stop=True,
                            )
                            # and the residual stream via the copy-out add
                            nc.vector.tensor_add(
                                out=acc[qt][:, ts(ch, CH)],
                                in0=ps, in1=x2[qt][:, ts(ch, CH)],
                            )
                        else:
                            nc.vector.tensor_add(
                                out=acc[qt][:, ts(ch, CH)],
                                in0=acc[qt][:, ts(ch, CH)], in1=ps,
                            )
                        if g == NG - 1:
                            # acc[qt] chunk finalized: store immediately
                            nc.sync.dma_start(
                                out=outd.ap()[ts(qt, P), ts(ch, CH)],
                                in_=acc[qt][:, ts(ch, CH)],
                            )

        psum.release()
        big.release()
        singles.release()

    nc.finalize()
    return nc


_NC_CACHE = None


def _get_nc():
    global _NC_CACHE
    if _NC_CACHE is None:
        _NC_CACHE = build_nc()
    return _NC_CACHE


def run(inputs, trace=False):
    """Run on 8 cores; returns (out [8,L,D], BassKernelResults)."""
    from concourse.bass_utils import run_bass_kernel_spmd

    nc = _get_nc()
    weights = {
        k: np.ascontiguousarray(np.asarray(inputs[k], dtype=np.float32))
        for k in ("ln1_g", "ln1_b", "Wq", "bq", "Wk", "bk", "Wv", "bv",
                  "Wo", "bo", "ln2_g", "ln2_b", "W1", "b1", "W2", "b2")
    }
    x = np.asarray(inputs["x"], dtype=np.float32)
    e_mask = np.asarray(inputs["e_mask"], dtype=np.int32)
    in_maps = []
    for b in range(B):
        m = dict(weights)
        m["x"] = np.ascontiguousarray(x[b])
        m["e_mask"] = np.ascontiguousarray(e_mask[b])
        in_maps.append(m)
    res = run_bass_kernel_spmd(nc, in_maps, core_ids=list(range(B)), trace=trace)
    out = np.stack([res.results[b]["out"] for b in range(B)], axis=0)
    return out, res


def kernel(**inputs):
    out, _ = run(inputs, trace=False)
    return out


# revision 27
# speedup vs baseline: 1.0714x; 1.0467x over previous
"""Trainium2 Bass kernel for a pre-LN transformer encoder layer.

Sharding: data-parallel over batch. B=8 batch elements -> 8 NeuronCores,
one full [L=1024, D=1024] encoder layer per core. No collectives.

Per-core dataflow (q = token index, d = feature index, k = key index):
  x [q,d] --LN1--> x1 [q,d] --PE transpose--> x1T [d,q] (bf16)
  V natural [k,d] (+ones col per head)  = matmul(lhsT=x1T tile, rhs=Wv rows)
  QT, KT [d,q]                          = matmul(lhsT=W col block, rhs=x1T)
  per head pair (chunk-major): ST [k,q] psum (row-packed across the two
            64-row head groups) -> ACT exp(s/8 + mask_bias) -> expS sbuf
            PV' accumulates [attnT | Z] over k tiles (ones-column trick)
            1/Z via custom-DVE approx reciprocal, replicated via a DRAM
            bounce -> attnT [d,q]
  attnproj [q,d] = matmul(lhsT=attnT tile, rhs=Wo rows); x2 = x + proj + bo
  LN2 -> x2n -> transpose -> x2nT [d,q]
  FFN1: hT [f,q] = matmul(lhsT=W1 col block, rhs=x2nT); ReLU+b1 fused in ACT
  FFN2: acc [q,d] += matmul(lhsT=hT tile, rhs=W2 rows) per f-group;
  + b2 once at the end.

Attention runs chunk-major (all 16 heads finish token-chunk 0 before
chunk 1) so the proj/LN2/FFN pipeline for the first half overlaps the
ACT-bound softmax of the second half. All matmul operands are bf16;
stats/softmax/residual arithmetic stays fp32.
"""

import numpy as np

import concourse.bass as bass
import concourse.tile as tile
from concourse import bacc, mybir
from concourse.bass import ds, ts
from concourse.masks import make_identity

B = 8
L = 1024
D = 1024
H = 16
DK = 64
F = 4096
EPS = 1e-6
NEG_INF = 1.0e9
P = 128
NQ = L // P            # 8 token tiles
ND = D // P            # 8 model-dim tiles
NF = F // P            # 32 ffn-dim tiles
CH = 512               # matmul moving free dim (one PSUM bank of fp32)
NCH = L // CH          # 2 chunks of tokens
QPC = CH // P          # 4 q-tiles per chunk
HPC = CH // DK         # 8 heads per 512-wide projection chunk
F_GROUP = 4            # f-tiles per FFN group
NG = NF // F_GROUP     # 8 groups

FP32 = mybir.dt.float32
MMD = mybir.dt.bfloat16   # matmul operand dtype
AF = mybir.ActivationFunctionType
OP = mybir.AluOpType


def build_nc():
    nc = bacc.Bacc("TRN2", target_bir_lowering=False, num_swdge_queues=4)

    xd = nc.dram_tensor("x", [L, D], FP32, kind="ExternalInput")
    maskd = nc.dram_tensor("e_mask", [1, L], mybir.dt.int32, kind="ExternalInput")
    ln1_g = nc.dram_tensor("ln1_g", [D], FP32, kind="ExternalInput")
    ln1_b = nc.dram_tensor("ln1_b", [D], FP32, kind="ExternalInput")
    wq = nc.dram_tensor("Wq", [D, D], FP32, kind="ExternalInput")
    bq = nc.dram_tensor("bq", [D], FP32, kind="ExternalInput")
    wk = nc.dram_tensor("Wk", [D, D], FP32, kind="ExternalInput")
    bk = nc.dram_tensor("bk", [D], FP32, kind="ExternalInput")
    wv = nc.dram_tensor("Wv", [D, D], FP32, kind="ExternalInput")
    bv = nc.dram_tensor("bv", [D], FP32, kind="ExternalInput")
    wo = nc.dram_tensor("Wo", [D, D], FP32, kind="ExternalInput")
    bo = nc.dram_tensor("bo", [D], FP32, kind="ExternalInput")
    ln2_g = nc.dram_tensor("ln2_g", [D], FP32, kind="ExternalInput")
    ln2_b = nc.dram_tensor("ln2_b", [D], FP32, kind="ExternalInput")
    w1 = nc.dram_tensor("W1", [D, F], FP32, kind="ExternalInput")
    b1 = nc.dram_tensor("b1", [F], FP32, kind="ExternalInput")
    w2 = nc.dram_tensor("W2", [F, D], FP32, kind="ExternalInput")
    b2 = nc.dram_tensor("b2", [D], FP32, kind="ExternalInput")
    outd = nc.dram_tensor("out", [L, D], FP32, kind="ExternalOutput")

    with tile.TileContext(nc) as tc:
        singles = tc.alloc_tile_pool(name="singles", bufs=1)
        big = tc.alloc_tile_pool(name="big", bufs=1)
        # single PSUM pool for the whole kernel: no pool-release barriers.
        # 4 (mm chains) + 2 (attention PV) + 2 (transposes) = 8 banks.
        psum = tc.alloc_tile_pool(name="psum", bufs=1, space="PSUM")

        def psum_mm():
            return psum.tile([P, CH], FP32, tag="mm", name="ps_mm", bufs=4)

        def big_tiles(shape, tagp, namep, dt=FP32):
            return [
                big.tile(shape, dt, tag=f"{tagp}{i}", name=f"{namep}{i}", bufs=1)
                for i in range(NQ)
            ]

        ident = singles.tile([P, P], MMD, name="ident")
        make_identity(nc, ident)
        eps_t = singles.tile([P, 1], FP32, name="eps_t")
        nc.vector.memset(eps_t, EPS)
        ones_h = singles.tile([P, H, 1], FP32, name="ones_h")
        nc.vector.memset(ones_h, 1.0)

        def bcast_load(pool, dram_vec, n, tag):
            """replicate a [n] DRAM vector across all 128 partitions."""
            t = pool.tile([P, n], FP32, tag=tag, name=tag, bufs=1)
            src = bass.AP(
                tensor=dram_vec.tensor,
                offset=dram_vec.offset,
                ap=[[0, P], [1, n]],
            )
            nc.sync.dma_start(out=t, in_=src)
            return t

        def col_load(dram_vec, ntiles, name):
            """[ntiles*128] DRAM vector -> [128, ntiles], col t = v[t*128:+128]."""
            t = singles.tile([P, ntiles], FP32, name=name)
            nc.sync.dma_start(out=t, in_=dram_vec.rearrange("(t p) -> p t", p=P))
            return t

        bq_c = col_load(bq.ap(), ND, "bq_c")
        bk_c = col_load(bk.ap(), ND, "bk_c")
        b1_c = col_load(b1.ap(), NF, "b1_c")

        # additive attention-mask bias per key position: (mask-1)*NEG_INF
        mask_i = singles.tile([P, NQ], mybir.dt.int32, name="mask_i")
        nc.sync.dma_start(out=mask_i, in_=maskd.ap()[0].rearrange("(t p) -> p t", p=P))
        mask_f = singles.tile([P, NQ], FP32, name="mask_f")
        nc.vector.tensor_copy(out=mask_f, in_=mask_i)
        ebias = singles.tile([P, NQ], FP32, name="ebias")
        nc.vector.tensor_scalar(
            out=ebias, in0=mask_f, scalar1=1.0, scalar2=NEG_INF,
            op0=OP.subtract, op1=OP.mult,
        )

        def layer_norm_tile(pool, x_t, g_bc, b_bc):
            stats = pool.tile([P, 2, 6], FP32, tag="ln_stats", name="ln_stats")
            xr = x_t.rearrange("p (s c) -> p s c", s=2)
            for s in range(2):
                nc.vector.bn_stats(out=stats[:, s, :], in_=xr[:, s, :])
            mv = pool.tile([P, 2], FP32, tag="ln_mv", name="ln_mv")
            nc.vector.bn_aggr(out=mv, in_=stats)
            rstd = pool.tile([P, 1], FP32, tag="ln_rstd", name="ln_rstd")
            nc.scalar.activation(out=rstd, in_=mv[:, 1:2], func=AF.Sqrt,
                                 bias=eps_t, scale=1.0)
            nc.vector.reciprocal(out=rstd, in_=rstd)
            xn = pool.tile([P, D], MMD, tag="ln_out", name="ln_out")
            xf = pool.tile([P, D], FP32, tag="ln_f32", name="ln_f32", bufs=1)
            nc.vector.tensor_scalar(
                out=xf, in0=x_t, scalar1=mv[:, 0:1], scalar2=rstd,
                op0=OP.subtract, op1=OP.mult,
            )
            nc.vector.tensor_mul(out=xf, in0=xf, in1=g_bc)
            nc.vector.tensor_add(out=xn, in0=xf, in1=b_bc)
            return xn

        def transpose_into(src_tile, qt, dst_tiles):
            """src natural [P, D] bf16 tile (token tile qt) -> dst [d,q] cols."""
            for dt in range(ND):
                pt = psum.tile([P, P], MMD, tag="tp", name="tp", bufs=2)
                nc.tensor.transpose(pt, src_tile[:, ts(dt, P)], ident)
                nc.vector.tensor_copy(out=dst_tiles[dt][:, ts(qt, P)], in_=pt)

        # persistent activations (tag groups; A is reused by x2nT later)
        x1T = big_tiles([P, L], "A", "x1T", MMD)
        qT = big_tiles([P, L], "B", "qT", MMD)
        kT = big_tiles([P, L], "C", "kT", MMD)
        attnT = big_tiles([P, L], "AT", "attnT", MMD)
        vn = [
            big.tile([P, H, DK + 1], MMD, tag=f"V{i}", name=f"vn{i}", bufs=1)
            for i in range(NQ)
        ]
        x2 = big_tiles([P, D], "X2", "x2", FP32)

        # weight prefetch pools allocated BEFORE phase 1 so the casting
        # DMAs start immediately (allocating them later would reuse ph1's
        # addresses and false-depend on LN1 finishing)
        ph4w = tc.alloc_tile_pool(name="ph4w", bufs=1)
        bo_bc = bcast_load(ph4w, bo.ap(), D, "bo_bc")
        wo_rows = []
        for dt in range(ND):
            wt = ph4w.tile([P, D], MMD, tag=f"wo_row{dt}",
                           name=f"wo_row{dt}", bufs=1)
            nc.gpsimd.dma_start(out=wt, in_=wo.ap()[ts(dt, P), :])
            wo_rows.append(wt)
        ph2v = tc.alloc_tile_pool(name="ph2v", bufs=1)
        bv_bc = bcast_load(ph2v, bv.ap(), D, "bv_bc")
        wv_rows = []
        for dt in range(ND):
            wt = ph2v.tile([P, D], MMD, tag=f"wv_row{dt}",
                           name=f"wv_row{dt}", bufs=1)
            nc.gpsimd.dma_start(out=wt, in_=wv.ap()[ts(dt, P), :])
            wv_rows.append(wt)
        for qt in range(NQ):
            nc.vector.tensor_copy(out=vn[qt][:, :, DK:DK + 1], in_=ones_h)

        # ---------- phase 1: LN1 + transpose ----------
        with tc.tile_pool(name="ph1", bufs=3) as ph1:
            g1_bc = bcast_load(ph1, ln1_g.ap(), D, "g1")
            b1ln_bc = bcast_load(ph1, ln1_b.ap(), D, "b1ln")
            for qt in range(NQ):
                x_t = ph1.tile([P, D], FP32, tag="x_in", name="x_in")
                nc.sync.dma_start(out=x_t, in_=xd.ap()[ts(qt, P), :])
                x1 = layer_norm_tile(ph1, x_t, g1_bc, b1ln_bc)
                transpose_into(x1, qt, x1T)

        # ---------- phase 2: V natural (+ones col) ----------
        if True:
            for qt in range(NQ):
                for ch in range(NCH):
                    ps = psum_mm()
                    for dt in range(ND):
                        nc.tensor.matmul(
                            ps, x1T[dt][:, ts(qt, P)],
                            wv_rows[dt][:, ts(ch, CH)],
                            start=(dt == 0), stop=(dt == ND - 1),
                        )
                    nc.vector.scalar_tensor_tensor(
                        out=vn[qt][:, ds(ch * HPC, HPC), 0:DK],
                        in0=ps.rearrange("p (h d) -> p h d", d=DK),
                        scalar=0.0,
                        in1=bv_bc[:, ts(ch, CH)].rearrange("p (h d) -> p h d", d=DK),
                        op0=OP.add, op1=OP.add,
                    )

        ph2v.release()

        # ---------- phases 3: QK + attention (chunk-major) ----------
        with tc.tile_pool(name="ph3", bufs=3) as ph3, \
             tc.tile_pool(name="ph3w", bufs=2) as ph3w, \
             tc.tile_pool(name="ph3d", bufs=3, space="DRAM") as ph3d:

            def emit_attention_pair_chunk(dt, ch):
                """S (row-packed across both heads of d-tile dt), exp, and
                the PV' accumulation step per k-tile, for token chunk ch.
                Streaming expS per k-tile keeps the S->exp->PV chain deep in
                flight with only [P, CH]-sized softmax buffers."""
                heads = (2 * dt, 2 * dt + 1)
                pa = {
                    h: psum.tile([P, CH], FP32, tag=f"pv{h % 2}",
                                 name="ps_a", bufs=1)
                    for h in heads
                }
                for kt in range(NQ):
                    es = {}
                    for h in heads:
                        rbase = (h % 2) * DK
                        ps = psum_mm()
                        nc.tensor.matmul(
                            ps,
                            kT[dt][rbase:rbase + DK, ts(kt, P)],
                            qT[dt][rbase:rbase + DK, ts(ch, CH)],
                            start=True, stop=True,
                        )
                        e = ph3.tile([P, CH], MMD, tag=f"expS{h % 2}",
                                     name="expS", bufs=3)
                        nc.scalar.activation(
                            out=e, in_=ps, func=AF.Exp,
                            bias=ebias[:, kt:kt + 1], scale=0.125,
                        )
                        es[h] = e
                    for h in heads:
                        nc.tensor.matmul(
                            pa[h][0:DK + 1, :],
                            vn[kt][:, h, :],
                            es[h],
                            start=(kt == 0), stop=(kt == NQ - 1),
                        )
                for h in heads:
                    rbase = (h % 2) * DK
                    # decouple the tail so the PV psum recycles after one copy
                    pv_sb = ph3.tile([P, CH], FP32, tag="pv_sb", name="pv_sb",
                                     bufs=2)
                    nc.vector.tensor_copy(out=pv_sb[0:DK + 1, :],
                                          in_=pa[h][0:DK + 1, :])
                    # ~51-ULP reciprocal of the Z row (full-tile custom-DVE
                    # op; sliced APs mislower). Replicate Z across partitions
                    # via a DRAM bounce (SBUF DMA sources need nonzero
                    # partition step, DRAM sources don't).
                    rzrow = ph3.tile([P, CH], FP32, tag="rzrow", name="rzrow", bufs=1)
                    nc.vector.reciprocal_approx_fast(out=rzrow, in_=pv_sb)
                    zscr = ph3d.tile([1, CH], FP32, tag="zscr", name="zscr")
                    nc.sync.dma_start(out=zscr, in_=rzrow[DK:DK + 1, :])
                    rzb = ph3.tile([DK, CH], FP32, tag="rzb", name="rzb", bufs=2)
                    nc.sync.dma_start(
                        out=rzb,
                        in_=bass.AP(
                            tensor=zscr.tensor, offset=zscr.offset,
                            ap=[[0, DK], [1, CH]],
                        ),
                    )
                    attn_h = ph3.tile([DK, CH], MMD, tag="attn_h", name="attn_h", bufs=2)
                    nc.vector.tensor_mul(out=attn_h, in0=pv_sb[0:DK, :], in1=rzb)
                    nc.sync.dma_start(
                        out=attnT[dt][rbase:rbase + DK, ts(ch, CH)], in_=attn_h
                    )

            for dt_out in range(ND):
                for (wmat, bias_c, dstT) in ((wq, bq_c, qT), (wk, bk_c, kT)):
                    wt = ph3w.tile([P, ND, P], MMD, tag="w_col", name="w_col")
                    nc.gpsimd.dma_start(
                        out=wt,
                        in_=wmat.ap().rearrange("(a p) b -> p a b", p=P)[
                            :, :, ts(dt_out, P)],
                    )
                    for ch in range(NCH):
                        ps = psum_mm()
                        for dt_in in range(ND):
                            nc.tensor.matmul(
                                ps, wt[:, dt_in, :],
                                x1T[dt_in][:, ts(ch, CH)],
                                start=(dt_in == 0), stop=(dt_in == ND - 1),
                            )
                        nc.scalar.activation(
                            out=dstT[dt_out][:, ts(ch, CH)], in_=ps,
                            func=AF.Identity, bias=bias_c[:, dt_out:dt_out + 1],
                            scale=1.0,
                        )
                emit_attention_pair_chunk(dt_out, 0)
            for dt_out in range(ND):
                emit_attention_pair_chunk(dt_out, 1)

            # ---------- phase 4+5: out-proj + residual + LN2 + transpose ----
            # Emitted inside the ph3 scope, chunk-major, so chunk-0 proj/LN2
            # overlaps the chunk-1 attention still in flight.
            x2nT = big_tiles([P, L], "A", "x2nT", MMD)  # reuses x1T slots
            with tc.tile_pool(name="ph4", bufs=2) as ph4:
                g2_bc = bcast_load(ph4, ln2_g.ap(), D, "g2")
                b2ln_bc = bcast_load(ph4, ln2_b.ap(), D, "b2ln")
                for ch in range(NCH):
                    for qi in range(QPC):
                        qt = ch * QPC + qi
                        x_t = ph4.tile([P, D], FP32, tag="x_again", name="x_again")
                        nc.sync.dma_start(out=x_t, in_=xd.ap()[ts(qt, P), :])
                        for oc in range(NCH):
                            ps = psum_mm()
                            for dt in range(ND):
                                nc.tensor.matmul(
                                    ps, attnT[dt][:, ts(qt, P)],
                                    wo_rows[dt][:, ts(oc, CH)],
                                    start=(dt == 0), stop=(dt == ND - 1),
                                )
                            nc.vector.tensor_add(
                                out=x2[qt][:, ts(oc, CH)], in0=ps,
                                in1=x_t[:, ts(oc, CH)],
                            )
                        nc.vector.tensor_add(out=x2[qt], in0=x2[qt], in1=bo_bc)
                        x2n = layer_norm_tile(ph4, x2[qt], g2_bc, b2ln_bc)
                        transpose_into(x2n, qt, x2nT)

        ph4w.release()

        # ---------- phase 6: FFN ----------
        acc = [
            big.tile([P, D], FP32, tag=f"V{i}", name=f"acc{i}", bufs=1)
            for i in range(NQ)
        ]

        with tc.tile_pool(name="ph6", bufs=1) as ph6, \
             tc.tile_pool(name="ph6w", bufs=2) as ph6w, \
             tc.tile_pool(name="ph6h", bufs=1) as ph6h:
            ones_row = ph6.tile([1, P], MMD, tag="ones_row", name="ones_row",
                                bufs=1)
            nc.vector.memset(ones_row, 1.0)
            b2_row = ph6.tile([1, D], MMD, tag="b2_row", name="b2_row", bufs=1)
            nc.gpsimd.dma_start(out=b2_row, in_=b2.ap().unsqueeze(0))
            w1r = w1.ap().rearrange("(a p) b -> p a b", p=P)
            for g in range(NG):
                hts = []
                w2_rows = []
                for fi in range(F_GROUP):
                    ft = g * F_GROUP + fi
                    w1t = ph6w.tile([P, ND, P], MMD, tag="w1_col", name="w1_col", bufs=4)
                    nc.gpsimd.dma_start(out=w1t, in_=w1r[:, :, ts(ft, P)])
                    w2t = ph6w.tile([P, D], MMD, tag=f"w2_row{fi}",
                                    name=f"w2_row{fi}", bufs=2)
                    nc.gpsimd.dma_start(out=w2t, in_=w2.ap()[ts(ft, P), :])
                    w2_rows.append(w2t)
                    ht = ph6h.tile([P, L], MMD, tag=f"ht{fi}",
                                   name=f"ht{fi}", bufs=2)
                    for ch in range(NCH):
                        ps = psum_mm()
                        for dt in range(ND):
                            nc.tensor.matmul(
                                ps, w1t[:, dt, :],
                                x2nT[dt][:, ts(ch, CH)],
                                start=(dt == 0), stop=(dt == ND - 1),
                            )
                        nc.scalar.activation(
                            out=ht[:, ts(ch, CH)], in_=ps, func=AF.Relu,
                            bias=b1_c[:, ft:ft + 1], scale=1.0,
                        )
                    hts.append(ht)
                for qt in range(NQ):
                    for ch in range(NCH):
                        ps = psum_mm()
                        for fi in range(F_GROUP):
                            nc.tensor.matmul(
                                ps, hts[fi][:, ts(qt, P)],
                                w2_rows[fi][:, ts(ch, CH)],
                                start=(fi == 0),
                                stop=(fi == F_GROUP - 1 and g != 0),
                            )
                        if g == 0:
                            # fold the fc2 bias in as a K=1 broadcast matmul
                            nc.tensor.matmul(
                                ps, ones_row, b2_row[:, ts(ch, CH)],
                                start=False, stop=True,
                            )
                            # and the residual stream via the copy-out add
                            nc.vector.tensor_add(
                                out=acc[qt][:, ts(ch, CH)],
                                in0=ps, in1=x2[qt][:, ts(ch, CH)],
                            )
                        else:
                            nc.vector.tensor_add(
                                out=acc[qt][:, ts(ch, CH)],
                                in0=acc[qt][:, ts(ch, CH)], in1=ps,
                            )
                        if g == NG - 1:
                            # acc[qt] chunk finalized: store immediately
                            nc.sync.dma_start(
                                out=outd.ap()[ts(qt, P), ts(ch, CH)],
                                in_=acc[qt][:, ts(ch, CH)],
                            )

        psum.release()
        big.release()
        singles.release()

    nc.finalize()
    return nc


_NC_CACHE = None


def _get_nc():
    global _NC_CACHE
    if _NC_CACHE is None:
        _NC_CACHE = build_nc()
    return _NC_CACHE


def run(inputs, trace=False):
    """Run on 8 cores; returns (out [8,L,D], BassKernelResults)."""
    from concourse.bass_utils import run_bass_kernel_spmd

    nc = _get_nc()
    weights = {
        k: np.ascontiguousarray(np.asarray(inputs[k], dtype=np.float32))
        for k in ("ln1_g", "ln1_b", "Wq", "bq", "Wk", "bk", "Wv", "bv",
                  "Wo", "bo", "ln2_g", "ln2_b", "W1", "b1", "W2", "b2")
    }
    x = np.asarray(inputs["x"], dtype=np.float32)
    e_mask = np.asarray(inputs["e_mask"], dtype=np.int32)
    in_maps = []
    for b in range(B):
        m = dict(weights)
        m["x"] = np.ascontiguousarray(x[b])
        m["e_mask"] = np.ascontiguousarray(e_mask[b])
        in_maps.append(m)
    last_err = None
    for _attempt in range(3):
        try:
            res = run_bass_kernel_spmd(
                nc, in_maps, core_ids=list(range(B)), trace=trace)
            break
        except Exception as e:  # transient NRT_EXEC_UNIT_UNRECOVERABLE wedges
            last_err = e
    else:
        raise last_err
    out = np.stack([res.results[b]["out"] for b in range(B)], axis=0)
    return out, res


def kernel(**inputs):
    out, _ = run(inputs, trace=False)
    return out


# revision 28
# speedup vs baseline: 1.0842x; 1.0120x over previous
"""Trainium2 Bass kernel for a pre-LN transformer encoder layer.

Sharding: data-parallel over batch. B=8 batch elements -> 8 NeuronCores,
one full [L=1024, D=1024] encoder layer per core. No collectives.

Per-core dataflow (q = token index, d = feature index, k = key index):
  x [q,d] --LN1--> x1 [q,d] --PE transpose--> x1T [d,q] (bf16)
  V natural [k,d] (+ones col per head)  = matmul(lhsT=x1T tile, rhs=Wv rows)
  QT, KT [d,q]                          = matmul(lhsT=W col block, rhs=x1T)
  per head pair (chunk-major): ST [k,q] psum (row-packed across the two
            64-row head groups) -> ACT exp(s/8 + mask_bias) -> expS sbuf
            PV' accumulates [attnT | Z] over k tiles (ones-column trick)
            1/Z via custom-DVE approx reciprocal, replicated via a DRAM
            bounce -> attnT [d,q]
  attnproj [q,d] = matmul(lhsT=attnT tile, rhs=Wo rows); x2 = x + proj + bo
  LN2 -> x2n -> transpose -> x2nT [d,q]
  FFN1: hT [f,q] = matmul(lhsT=W1 col block, rhs=x2nT); ReLU+b1 fused in ACT
  FFN2: acc [q,d] += matmul(lhsT=hT tile, rhs=W2 rows) per f-group;
  + b2 once at the end.

Attention runs chunk-major (all 16 heads finish token-chunk 0 before
chunk 1) so the proj/LN2/FFN pipeline for the first half overlaps the
ACT-bound softmax of the second half. All matmul operands are bf16;
stats/softmax/residual arithmetic stays fp32.
"""

import numpy as np

import concourse.bass as bass
import concourse.tile as tile
from concourse import bacc, mybir
from concourse.bass import ds, ts
from concourse.masks import make_identity

B = 8
L = 1024
D = 1024
H = 16
DK = 64
F = 4096
EPS = 1e-6
NEG_INF = 1.0e9
P = 128
NQ = L // P            # 8 token tiles
ND = D // P            # 8 model-dim tiles
NF = F // P            # 32 ffn-dim tiles
CH = 512               # matmul moving free dim (one PSUM bank of fp32)
NCH = L // CH          # 2 chunks of tokens
QPC = CH // P          # 4 q-tiles per chunk
HPC = CH // DK         # 8 heads per 512-wide projection chunk
F_GROUP = 4            # f-tiles per FFN group
NG = NF // F_GROUP     # 8 groups

FP32 = mybir.dt.float32
MMD = mybir.dt.bfloat16   # matmul operand dtype
AF = mybir.ActivationFunctionType
OP = mybir.AluOpType


def build_nc():
    nc = bacc.Bacc("TRN2", target_bir_lowering=False, num_swdge_queues=4)

    xd = nc.dram_tensor("x", [L, D], FP32, kind="ExternalInput")
    maskd = nc.dram_tensor("e_mask", [1, L], mybir.dt.int32, kind="ExternalInput")
    ln1_g = nc.dram_tensor("ln1_g", [D], FP32, kind="ExternalInput")
    ln1_b = nc.dram_tensor("ln1_b", [D], FP32, kind="ExternalInput")
    wq = nc.dram_tensor("Wq", [D, D], FP32, kind="ExternalInput")
    bq = nc.dram_tensor("bq", [D], FP32, kind="ExternalInput")
    wk = nc.dram_tensor("Wk", [D, D], FP32, kind="ExternalInput")
    bk = nc.dram_tensor("bk", [D], FP32, kind="ExternalInput")
    wv = nc.dram_tensor("Wv", [D, D], FP32, kind="ExternalInput")
    bv = nc.dram_tensor("bv", [D], FP32, kind="ExternalInput")
    wo = nc.dram_tensor("Wo", [D, D], FP32, kind="ExternalInput")
    bo = nc.dram_tensor("bo", [D], FP32, kind="ExternalInput")
    ln2_g = nc.dram_tensor("ln2_g", [D], FP32, kind="ExternalInput")
    ln2_b = nc.dram_tensor("ln2_b", [D], FP32, kind="ExternalInput")
    w1 = nc.dram_tensor("W1", [D, F], FP32, kind="ExternalInput")
    b1 = nc.dram_tensor("b1", [F], FP32, kind="ExternalInput")
    w2 = nc.dram_tensor("W2", [F, D], FP32, kind="ExternalInput")
    b2 = nc.dram_tensor("b2", [D], FP32, kind="ExternalInput")
    outd = nc.dram_tensor("out", [L, D], FP32, kind="ExternalOutput")

    with tile.TileContext(nc) as tc:
        singles = tc.alloc_tile_pool(name="singles", bufs=1)
        big = tc.alloc_tile_pool(name="big", bufs=1)
        # single PSUM pool for the whole kernel: no pool-release barriers.
        # 4 (mm chains) + 2 (attention PV) + 2 (transposes) = 8 banks.
        psum = tc.alloc_tile_pool(name="psum", bufs=1, space="PSUM")

        def psum_mm():
            return psum.tile([P, CH], FP32, tag="mm", name="ps_mm", bufs=4)

        def big_tiles(shape, tagp, namep, dt=FP32):
            return [
                big.tile(shape, dt, tag=f"{tagp}{i}", name=f"{namep}{i}", bufs=1)
                for i in range(NQ)
            ]

        ident = singles.tile([P, P], MMD, name="ident")
        make_identity(nc, ident)
        eps_t = singles.tile([P, 1], FP32, name="eps_t")
        nc.vector.memset(eps_t, EPS)
        ones_h = singles.tile([P, H, 1], FP32, name="ones_h")
        nc.vector.memset(ones_h, 1.0)

        def bcast_load(pool, dram_vec, n, tag):
            """replicate a [n] DRAM vector across all 128 partitions."""
            t = pool.tile([P, n], FP32, tag=tag, name=tag, bufs=1)
            src = bass.AP(
                tensor=dram_vec.tensor,
                offset=dram_vec.offset,
                ap=[[0, P], [1, n]],
            )
            nc.sync.dma_start(out=t, in_=src)
            return t

        def col_load(dram_vec, ntiles, name):
            """[ntiles*128] DRAM vector -> [128, ntiles], col t = v[t*128:+128]."""
            t = singles.tile([P, ntiles], FP32, name=name)
            nc.sync.dma_start(out=t, in_=dram_vec.rearrange("(t p) -> p t", p=P))
            return t

        bq_c = col_load(bq.ap(), ND, "bq_c")
        bk_c = col_load(bk.ap(), ND, "bk_c")
        b1_c = col_load(b1.ap(), NF, "b1_c")

        # additive attention-mask bias per key position: (mask-1)*NEG_INF
        mask_i = singles.tile([P, NQ], mybir.dt.int32, name="mask_i")
        nc.sync.dma_start(out=mask_i, in_=maskd.ap()[0].rearrange("(t p) -> p t", p=P))
        mask_f = singles.tile([P, NQ], FP32, name="mask_f")
        nc.vector.tensor_copy(out=mask_f, in_=mask_i)
        ebias = singles.tile([P, NQ], FP32, name="ebias")
        nc.vector.tensor_scalar(
            out=ebias, in0=mask_f, scalar1=1.0, scalar2=NEG_INF,
            op0=OP.subtract, op1=OP.mult,
        )

        def layer_norm_tile(pool, x_t, g_bc, b_bc):
            stats = pool.tile([P, 2, 6], FP32, tag="ln_stats", name="ln_stats")
            xr = x_t.rearrange("p (s c) -> p s c", s=2)
            for s in range(2):
                nc.vector.bn_stats(out=stats[:, s, :], in_=xr[:, s, :])
            mv = pool.tile([P, 2], FP32, tag="ln_mv", name="ln_mv")
            nc.vector.bn_aggr(out=mv, in_=stats)
            rstd = pool.tile([P, 1], FP32, tag="ln_rstd", name="ln_rstd")
            nc.scalar.activation(out=rstd, in_=mv[:, 1:2], func=AF.Sqrt,
                                 bias=eps_t, scale=1.0)
            nc.vector.reciprocal(out=rstd, in_=rstd)
            xn = pool.tile([P, D], MMD, tag="ln_out", name="ln_out")
            xf = pool.tile([P, D], FP32, tag="ln_f32", name="ln_f32", bufs=1)
            nc.vector.tensor_scalar(
                out=xf, in0=x_t, scalar1=mv[:, 0:1], scalar2=rstd,
                op0=OP.subtract, op1=OP.mult,
            )
            nc.vector.tensor_mul(out=xf, in0=xf, in1=g_bc)
            nc.vector.tensor_add(out=xn, in0=xf, in1=b_bc)
            return xn

        def transpose_into(src_tile, qt, dst_tiles):
            """src natural [P, D] bf16 tile (token tile qt) -> dst [d,q] cols."""
            for dt in range(ND):
                pt = psum.tile([P, P], MMD, tag="tp", name="tp", bufs=2)
                nc.tensor.transpose(pt, src_tile[:, ts(dt, P)], ident)
                nc.vector.tensor_copy(out=dst_tiles[dt][:, ts(qt, P)], in_=pt)

        # persistent activations (tag groups; A is reused by x2nT later)
        x1T = big_tiles([P, L], "A", "x1T", MMD)
        qT = big_tiles([P, L], "B", "qT", MMD)
        kT = big_tiles([P, L], "C", "kT", MMD)
        attnT = big_tiles([P, L], "AT", "attnT", MMD)
        vn = [
            big.tile([P, H, DK + 1], MMD, tag=f"V{i}", name=f"vn{i}", bufs=1)
            for i in range(NQ)
        ]
        x2 = big_tiles([P, D], "X2", "x2", FP32)

        # weight prefetch pools allocated BEFORE phase 1 so the casting
        # DMAs start immediately (allocating them later would reuse ph1's
        # addresses and false-depend on LN1 finishing)
        ph4w = tc.alloc_tile_pool(name="ph4w", bufs=1)
        bo_bc = bcast_load(ph4w, bo.ap(), D, "bo_bc")
        wo_rows = []
        for dt in range(ND):
            wt = ph4w.tile([P, D], MMD, tag=f"wo_row{dt}",
                           name=f"wo_row{dt}", bufs=1)
            nc.gpsimd.dma_start(out=wt, in_=wo.ap()[ts(dt, P), :])
            wo_rows.append(wt)
        ph2v = tc.alloc_tile_pool(name="ph2v", bufs=1)
        bv_bc = bcast_load(ph2v, bv.ap(), D, "bv_bc")
        wv_rows = []
        for dt in range(ND):
            wt = ph2v.tile([P, D], MMD, tag=f"wv_row{dt}",
                           name=f"wv_row{dt}", bufs=1)
            nc.gpsimd.dma_start(out=wt, in_=wv.ap()[ts(dt, P), :])
            wv_rows.append(wt)
        for qt in range(NQ):
            nc.vector.tensor_copy(out=vn[qt][:, :, DK:DK + 1], in_=ones_h)

        # ---------- phase 1: LN1 + transpose ----------
        with tc.tile_pool(name="ph1", bufs=3) as ph1:
            g1_bc = bcast_load(ph1, ln1_g.ap(), D, "g1")
            b1ln_bc = bcast_load(ph1, ln1_b.ap(), D, "b1ln")
            for qt in range(NQ):
                x_t = ph1.tile([P, D], FP32, tag="x_in", name="x_in")
                nc.sync.dma_start(out=x_t, in_=xd.ap()[ts(qt, P), :])
                x1 = layer_norm_tile(ph1, x_t, g1_bc, b1ln_bc)
                transpose_into(x1, qt, x1T)

        # ---------- phase 2: V natural (+ones col) ----------
        if True:
            for qt in range(NQ):
                for ch in range(NCH):
                    ps = psum_mm()
                    for dt in range(ND):
                        nc.tensor.matmul(
                            ps, x1T[dt][:, ts(qt, P)],
                            wv_rows[dt][:, ts(ch, CH)],
                            start=(dt == 0), stop=(dt == ND - 1),
                        )
                    nc.vector.scalar_tensor_tensor(
                        out=vn[qt][:, ds(ch * HPC, HPC), 0:DK],
                        in0=ps.rearrange("p (h d) -> p h d", d=DK),
                        scalar=0.0,
                        in1=bv_bc[:, ts(ch, CH)].rearrange("p (h d) -> p h d", d=DK),
                        op0=OP.add, op1=OP.add,
                    )

        ph2v.release()

        # ---------- phases 3: QK + attention (chunk-major) ----------
        with tc.tile_pool(name="ph3", bufs=3) as ph3, \
             tc.tile_pool(name="ph3w", bufs=2) as ph3w, \
             tc.tile_pool(name="ph3d", bufs=3, space="DRAM") as ph3d:

            def emit_attention_pair_chunk(dt, ch):
                """S (row-packed across both heads of d-tile dt), exp, and
                the PV' accumulation step per k-tile, for token chunk ch.
                Streaming expS per k-tile keeps the S->exp->PV chain deep in
                flight with only [P, CH]-sized softmax buffers."""
                heads = (2 * dt, 2 * dt + 1)
                pa = {
                    h: psum.tile([P, CH], FP32, tag=f"pv{h % 2}",
                                 name="ps_a", bufs=1)
                    for h in heads
                }
                for kt in range(NQ):
                    es = {}
                    for h in heads:
                        rbase = (h % 2) * DK
                        ps = psum_mm()
                        nc.tensor.matmul(
                            ps,
                            kT[dt][rbase:rbase + DK, ts(kt, P)],
                            qT[dt][rbase:rbase + DK, ts(ch, CH)],
                            start=True, stop=True,
                        )
                        e = ph3.tile([P, CH], MMD, tag=f"expS{h % 2}",
                                     name="expS", bufs=3)
                        nc.scalar.activation(
                            out=e, in_=ps, func=AF.Exp,
                            bias=ebias[:, kt:kt + 1], scale=0.125,
                        )
                        es[h] = e
                    for h in heads:
                        nc.tensor.matmul(
                            pa[h][0:DK + 1, :],
                            vn[kt][:, h, :],
                            es[h],
                            start=(kt == 0), stop=(kt == NQ - 1),
                        )
                for h in heads:
                    rbase = (h % 2) * DK
                    # decouple the tail so the PV psum recycles after one copy
                    pv_sb = ph3.tile([P, CH], FP32, tag="pv_sb", name="pv_sb",
                                     bufs=2)
                    nc.vector.tensor_copy(out=pv_sb[0:DK + 1, :],
                                          in_=pa[h][0:DK + 1, :])
                    # ~51-ULP reciprocal of the Z row (full-tile custom-DVE
                    # op; sliced APs mislower). Replicate Z across partitions
                    # via a DRAM bounce (SBUF DMA sources need nonzero
                    # partition step, DRAM sources don't).
                    rzrow = ph3.tile([P, CH], FP32, tag="rzrow", name="rzrow", bufs=1)
                    nc.vector.reciprocal_approx_fast(out=rzrow, in_=pv_sb)
                    zscr = ph3d.tile([1, CH], FP32, tag="zscr", name="zscr")
                    nc.sync.dma_start(out=zscr, in_=rzrow[DK:DK + 1, :])
                    rzb = ph3.tile([DK, CH], FP32, tag="rzb", name="rzb", bufs=2)
                    nc.sync.dma_start(
                        out=rzb,
                        in_=bass.AP(
                            tensor=zscr.tensor, offset=zscr.offset,
                            ap=[[0, DK], [1, CH]],
                        ),
                    )
                    attn_h = ph3.tile([DK, CH], MMD, tag="attn_h", name="attn_h", bufs=2)
                    nc.vector.tensor_mul(out=attn_h, in0=pv_sb[0:DK, :], in1=rzb)
                    nc.sync.dma_start(
                        out=attnT[dt][rbase:rbase + DK, ts(ch, CH)], in_=attn_h
                    )

            for dt_out in range(ND):
                for (wmat, bias_c, dstT) in ((wq, bq_c, qT), (wk, bk_c, kT)):
                    wt = ph3w.tile([P, ND, P], MMD, tag="w_col", name="w_col")
                    nc.gpsimd.dma_start(
                        out=wt,
                        in_=wmat.ap().rearrange("(a p) b -> p a b", p=P)[
                            :, :, ts(dt_out, P)],
                    )
                    for ch in range(NCH):
                        ps = psum_mm()
                        for dt_in in range(ND):
                            nc.tensor.matmul(
                                ps, wt[:, dt_in, :],
                                x1T[dt_in][:, ts(ch, CH)],
                                start=(dt_in == 0), stop=(dt_in == ND - 1),
                            )
                        nc.scalar.activation(
                            out=dstT[dt_out][:, ts(ch, CH)], in_=ps,
                            func=AF.Identity, bias=bias_c[:, dt_out:dt_out + 1],
                            scale=1.0,
                        )
                emit_attention_pair_chunk(dt_out, 0)
            for dt_out in range(ND):
                emit_attention_pair_chunk(dt_out, 1)

            # ---------- phase 4+5: out-proj + residual + LN2 + transpose ----
            # Emitted inside the ph3 scope, chunk-major, so chunk-0 proj/LN2
            # overlaps the chunk-1 attention still in flight.
            x2nT = big_tiles([P, L], "A", "x2nT", MMD)  # reuses x1T slots
            with tc.tile_pool(name="ph4", bufs=2) as ph4:
                g2_bc = bcast_load(ph4, ln2_g.ap(), D, "g2")
                b2ln_bc = bcast_load(ph4, ln2_b.ap(), D, "b2ln")
                for ch in range(NCH):
                    for qi in range(QPC):
                        qt = ch * QPC + qi
                        x_t = ph4.tile([P, D], FP32, tag="x_again", name="x_again")
                        nc.sync.dma_start(out=x_t, in_=xd.ap()[ts(qt, P), :])
                        for oc in range(NCH):
                            ps = psum_mm()
                            for dt in range(ND):
                                nc.tensor.matmul(
                                    ps, attnT[dt][:, ts(qt, P)],
                                    wo_rows[dt][:, ts(oc, CH)],
                                    start=(dt == 0), stop=(dt == ND - 1),
                                )
                            nc.vector.tensor_add(
                                out=x2[qt][:, ts(oc, CH)], in0=ps,
                                in1=x_t[:, ts(oc, CH)],
                            )
                        nc.vector.tensor_add(out=x2[qt], in0=x2[qt], in1=bo_bc)
                        x2n = layer_norm_tile(ph4, x2[qt], g2_bc, b2ln_bc)
                        transpose_into(x2n, qt, x2nT)

        # ---------- phase 6: FFN ----------
        acc = [
            big.tile([P, D], FP32, tag=f"V{i}", name=f"acc{i}", bufs=1)
            for i in range(NQ)
        ]

        with tc.tile_pool(name="ph6", bufs=1) as ph6, \
             tc.tile_pool(name="ph6w", bufs=2) as ph6w, \
             tc.tile_pool(name="ph6h", bufs=1) as ph6h:
            ones_row = ph6.tile([1, P], MMD, tag="ones_row", name="ones_row",
                                bufs=1)
            nc.vector.memset(ones_row, 1.0)
            b2_row = ph6.tile([1, D], MMD, tag="b2_row", name="b2_row", bufs=1)
            nc.gpsimd.dma_start(out=b2_row, in_=b2.ap().unsqueeze(0))
            w1r = w1.ap().rearrange("(a p) b -> p a b", p=P)
            for g in range(NG):
                hts = []
                w2_rows = []
                for fi in range(F_GROUP):
                    ft = g * F_GROUP + fi
                    w1t = ph6w.tile([P, ND, P], MMD, tag="w1_col", name="w1_col", bufs=4)
                    nc.gpsimd.dma_start(out=w1t, in_=w1r[:, :, ts(ft, P)])
                    w2t = ph4w.tile([P, D], MMD,
                                    tag=f"wo_row{(g % 2) * F_GROUP + fi}",
                                    name=f"w2_row{fi}", bufs=1)
                    nc.gpsimd.dma_start(out=w2t, in_=w2.ap()[ts(ft, P), :])
                    w2_rows.append(w2t)
                    ht = ph6h.tile([P, L], MMD, tag=f"ht{fi}",
                                   name=f"ht{fi}", bufs=4)
                    for ch in range(NCH):
                        ps = psum_mm()
                        for dt in range(ND):
                            nc.tensor.matmul(
                                ps, w1t[:, dt, :],
                                x2nT[dt][:, ts(ch, CH)],
                                start=(dt == 0), stop=(dt == ND - 1),
                            )
                        nc.scalar.activation(
                            out=ht[:, ts(ch, CH)], in_=ps, func=AF.Relu,
                            bias=b1_c[:, ft:ft + 1], scale=1.0,
                        )
                    hts.append(ht)
                for qt in range(NQ):
                    for ch in range(NCH):
                        ps = psum_mm()
                        for fi in range(F_GROUP):
                            nc.tensor.matmul(
                                ps, hts[fi][:, ts(qt, P)],
                                w2_rows[fi][:, ts(ch, CH)],
                                start=(fi == 0),
                                stop=(fi == F_GROUP - 1 and g != 0),
                            )
                        if g == 0:
                            # fold the fc2 bias in as a K=1 broadcast matmul
                            nc.tensor.matmul(
                                ps, ones_row, b2_row[:, ts(ch, CH)],
                                start=False, stop=True,
                            )
                            # and the residual stream via the copy-out add
                            nc.vector.tensor_add(
                                out=acc[qt][:, ts(ch, CH)],
                                in0=ps, in1=x2[qt][:, ts(ch, CH)],
                            )
                        else:
                            nc.vector.tensor_add(
                                out=acc[qt][:, ts(ch, CH)],
                                in0=acc[qt][:, ts(ch, CH)], in1=ps,
                            )
                        if g == NG - 1:
                            # acc[qt] chunk finalized: store immediately
                            nc.sync.dma_start(
                                out=outd.ap()[ts(qt, P), ts(ch, CH)],
                                in_=acc[qt][:, ts(ch, CH)],
                            )

        ph4w.release()
        psum.release()
        big.release()
        singles.release()

    nc.finalize()
    return nc


_NC_CACHE = None


def _get_nc():
    global _NC_CACHE
    if _NC_CACHE is None:
        _NC_CACHE = build_nc()
    return _NC_CACHE


def run(inputs, trace=False):
    """Run on 8 cores; returns (out [8,L,D], BassKernelResults)."""
    from concourse.bass_utils import run_bass_kernel_spmd

    nc = _get_nc()
    weights = {
        k: np.ascontiguousarray(np.asarray(inputs[k], dtype=np.float32))
        for k in ("ln1_g", "ln1_b", "Wq", "bq", "Wk", "bk", "Wv", "bv",
                  "Wo", "bo", "ln2_g", "ln2_b", "W1", "b1", "W2", "b2")
    }
    x = np.asarray(inputs["x"], dtype=np.float32)
    e_mask = np.asarray(inputs["e_mask"], dtype=np.int32)
    in_maps = []
    for b in range(B):
        m = dict(weights)
        m["x"] = np.ascontiguousarray(x[b])
        m["e_mask"] = np.ascontiguousarray(e_mask[b])
        in_maps.append(m)
    last_err = None
    for _attempt in range(3):
        try:
            res = run_bass_kernel_spmd(
                nc, in_maps, core_ids=list(range(B)), trace=trace)
            break
        except Exception as e:  # transient NRT_EXEC_UNIT_UNRECOVERABLE wedges
            last_err = e
    else:
        raise last_err
    out = np.stack([res.results[b]["out"] for b in range(B)], axis=0)
    return out, res


def kernel(**inputs):
    out, _ = run(inputs, trace=False)
    return out
